# revision 1
# baseline (speedup 1.0000x reference)
"""HGConv fused kernel for one TRN2 chip (8 NeuronCores), SPMD via Bass/Tile.

Hardcoded for M=16384 nodes, E=4096 hyperedges, D=300, N_CAT=3, 8 cores.

  - Shard the node axis m: core c gets node_feats rows [2048c, 2048(c+1))
    and the matching inc_mat rows.  Phase 1 computes the partial
    IX_c = inc_c.T @ X_c (4096, 300) with inc tiles stationary on the PE.
  - ReduceScatter(add) turns the partials into the true IX = inc.T @ X,
    e-sharded: core c owns edges [512c, 512(c+1)).
  - Local tail per core: edge_att = IX @ W_att (reassociated from
    inc.T @ (X @ W_att)), softmax over d, ef = (IX * attn) @ W_proj,
    residual mix with edge_feats, scores = ef2 @ ec_W_att, locally
    stabilized exp, G = ef2 @ ec_W_proj, partial pooled vector
    p2 = sum_e exp_e * G[e, :].
  - AllGather of the per-core (p2, z, m) partials (304 floats); every core
    redundantly combines them (global softmax over edges) and applies the
    two tiny projections to produce the (3,) logits.
"""

import sys

for _p in ("/opt/trn_rl_repo", "/opt/pypackages"):
    if _p not in sys.path:
        sys.path.append(_p)

import numpy as np

import concourse.bacc as bacc
import concourse.tile as tile
from concourse import masks, mybir
from concourse.bass_utils import run_bass_kernel_spmd

F32 = mybir.dt.float32
F32R = mybir.dt.float32r
BF16 = mybir.dt.bfloat16
AX = mybir.AxisListType
OP = mybir.AluOpType
AF = mybir.ActivationFunctionType

NCORES = 8
M, E, D, NCAT = 16384, 4096, 300, 3
M_SH = M // NCORES          # 2048 nodes per core
E_SH = E // NCORES          # 512 edges per core (tail shard)
MT = M_SH // 128            # 16 m-tiles per core
ET_SH = E_SH // 128         # 4 e-tiles per core
DCH = (128, 128, 44)        # d split into partition chunks
DOF = (0, 128, 256)
E_BLK = 1024                # phase-1 e block (8 psum banks)
N_EBLK = E // E_BLK
E_SUB = E_BLK // 128


def _build(alpha: float, mode: str):
    nc = bacc.Bacc("TRN2", target_bir_lowering=False, debug=False,
                   num_devices=NCORES)
    in_dt = BF16 if mode == "bf16" else F32
    x_d = nc.dram_tensor("x", [M_SH, D], in_dt, kind="ExternalInput")
    inc_d = nc.dram_tensor("inc", [M_SH, E], in_dt, kind="ExternalInput")
    ef_d = nc.dram_tensor("efeat", [E_SH, D], F32, kind="ExternalInput")
    watt_d = nc.dram_tensor("watt", [D, D], F32, kind="ExternalInput")
    wproj_d = nc.dram_tensor("wproj", [D, D], F32, kind="ExternalInput")
    ecwatt_d = nc.dram_tensor("ecwatt", [D, 1], F32, kind="ExternalInput")
    ecwproj_d = nc.dram_tensor("ecwproj", [D, D], F32, kind="ExternalInput")
    ecb_d = nc.dram_tensor("ecb", [D], F32, kind="ExternalInput")
    fcw_d = nc.dram_tensor("fcw", [D, NCAT], F32, kind="ExternalInput")
    fcb_d = nc.dram_tensor("fcb", [NCAT], F32, kind="ExternalInput")
    out_d = nc.dram_tensor("out", [1, NCAT], F32, kind="ExternalOutput")

    groups = [list(range(NCORES))]

    rdt = {"f32": F32, "f32r": F32R, "bf16": BF16}[mode]
    e_blk = 2048 if mode == "bf16" else 1024
    n_eblk = E // e_blk
    e_sub = e_blk // 128

    def mm(out, lhsT, rhs, start, stop):
        nc.tensor.matmul(out, lhsT, rhs, start=start, stop=stop)

    def rsrc(ap):
        return ap.bitcast(F32R) if mode == "f32r" else ap

    with tile.TileContext(nc) as tc, \
         tc.tile_pool(name="sb", bufs=1) as sb, \
         tc.tile_pool(name="dram", bufs=1, space="DRAM") as dram:

        p_chunks = [dram.tile([1024, D], F32, name=f"p_chunk{k}")
                    for k in range(4)]          # RS inputs (partial IX)
        r_ks = [dram.tile([128, D], F32, name=f"r_k{k}")
                for k in range(4)]              # RS outputs (my 128 edges)
        pk_dram = dram.tile([304], F32)         # AG input
        gath = dram.tile([NCORES, 304], F32)    # AG output

        # ---------- phase 1: IX partial = inc_c.T @ X_c ----------
        x_sb = sb.tile([128, MT, D], rdt)
        nc.sync.dma_start(x_sb[:], rsrc(x_d.ap().rearrange("(t p) d -> p t d",
                                                           p=128)))
        with tc.tile_pool(name="incp", bufs=MT + 8) as incp, \
             tc.tile_pool(name="stg", bufs=8) as stg, \
             tc.tile_pool(name="pp1", bufs=8, space="PSUM") as pp1:
            for blk in range(n_eblk):
                inc_sb = [incp.tile([128, e_blk], rdt, tag="inc",
                                    name=f"inc_b{blk}_m{m}")
                          for m in range(MT)]
                for m in range(MT):
                    eng = nc.sync if m % 2 == 0 else nc.scalar
                    eng.dma_start(
                        inc_sb[m][:],
                        rsrc(inc_d[m * 128:(m + 1) * 128,
                                   blk * e_blk:(blk + 1) * e_blk]))
                for es in range(e_sub):
                    acc = pp1.tile([128, D], F32, tag="p1")
                    for m in range(MT):
                        mm(acc[:], inc_sb[m][:, es * 128:(es + 1) * 128],
                           x_sb[:, m, :], start=(m == 0), stop=(m == MT - 1))
                    stage = stg.tile([128, D], F32, tag="stage",
                                     name=f"stage_{blk}_{es}")
                    nc.vector.tensor_copy(stage[:], acc[:])
                    eg = blk * e_blk + es * 128        # global edge offset
                    k, row = eg // 1024, eg % 1024
                    nc.gpsimd.dma_start(p_chunks[k][row:row + 128, :],
                                        stage[:])
                    # phase 2 (chunked, overlapped): as soon as chunk k is
                    # fully written, ReduceScatter it while the next block
                    # computes.
                    if row == 1024 - 128:
                        nc.gpsimd.collective_compute(
                            "ReduceScatter", OP.add, replica_groups=groups,
                            ins=[p_chunks[k].opt()], outs=[r_ks[k].opt()])

        # ---------- small weights / constants ----------
        watt_sb = sb.tile([128, 3, D], F32)
        wproj_sb = sb.tile([128, 3, D], F32)
        ecwproj_sb = sb.tile([128, 3, D], F32)
        fcw_sb = sb.tile([128, 3, NCAT], F32)
        ecwatt_sb = sb.tile([128, 3, 1], F32)
        for i, (c, o) in enumerate(zip(DCH, DOF)):
            nc.sync.dma_start(watt_sb[:c, i, :], watt_d[o:o + c, :])
            nc.sync.dma_start(wproj_sb[:c, i, :], wproj_d[o:o + c, :])
            nc.sync.dma_start(ecwproj_sb[:c, i, :], ecwproj_d[o:o + c, :])
            nc.sync.dma_start(fcw_sb[:c, i, :], fcw_d[o:o + c, :])
            nc.sync.dma_start(ecwatt_sb[:c, i, :], ecwatt_d[o:o + c, :])
        ecb_sb = sb.tile([1, D], F32)
        nc.sync.dma_start(ecb_sb[:], ecb_d.ap().rearrange("(o d) -> o d", o=1))
        fcb_sb = sb.tile([1, NCAT], F32)
        nc.sync.dma_start(fcb_sb[:], fcb_d.ap().rearrange("(o d) -> o d", o=1))
        ident = sb.tile([128, 128], F32)
        masks.make_identity(nc, ident[:])
        efeat_sb = sb.tile([128, ET_SH, D], F32)
        nc.sync.dma_start(efeat_sb[:],
                          ef_d.ap().rearrange("(t p) d -> p t d", p=128))

        # ---------- phase 3: local tail on this core's 512 edges ----------
        ix_sb = sb.tile([128, ET_SH, D], F32)
        for k in range(4):
            nc.sync.dma_start(ix_sb[:, k, :], r_ks[k][:])

        with tc.tile_pool(name="pp2", bufs=4, space="PSUM") as pp:

            def transpose_512xD(src_sb, dstT_sb):
                # src (128, 4, 300) [e-part] -> dstT (128, 3, 512) [d-part]
                for et in range(ET_SH):
                    for i, (c, o) in enumerate(zip(DCH, DOF)):
                        tp = pp.tile([128, 128], F32, tag="ps")
                        nc.tensor.transpose(tp[:c, :128],
                                            src_sb[:, et, o:o + c], ident[:])
                        nc.scalar.copy(
                            dstT_sb[:c, i, et * 128:(et + 1) * 128],
                            tp[:c, :128])

            ixT_sb = sb.tile([128, 3, E_SH], F32)
            transpose_512xD(ix_sb, ixT_sb)

            # edge_att = IX @ W_att; softmax over d; ef = IX * attn
            ef2_sb = sb.tile([128, ET_SH, D], F32)
            stat_sb = sb.tile([128, ET_SH, 4], F32)
            for et in range(ET_SH):
                att = pp.tile([128, D], F32, tag="ps")
                for i, c in enumerate(DCH):
                    mm(att[:], ixT_sb[:c, i, et * 128:(et + 1) * 128],
                       watt_sb[:c, i, :], start=(i == 0), stop=(i == 2))
                nmax = stat_sb[:, et, 0:1]
                nc.vector.tensor_reduce(nmax, att[:], axis=AX.X, op=OP.max,
                                        negate=True)
                ex = pp.tile([128, D], F32, tag="ps")
                rsum = stat_sb[:, et, 1:2]
                nc.scalar.activation(ex[:], att[:], AF.Exp, bias=nmax,
                                     scale=1.0, accum_out=rsum)
                rcp = stat_sb[:, et, 2:3]
                nc.vector.reciprocal(rcp, rsum)
                nc.vector.scalar_tensor_tensor(
                    ef2_sb[:, et, :], ex[:], rcp, ix_sb[:, et, :],
                    op0=OP.mult, op1=OP.mult)

            efT_sb = sb.tile([128, 3, E_SH], F32)
            transpose_512xD(ef2_sb, efT_sb)

            # ef2 = alpha * edge_feats + (1 - alpha) * (ef @ W_proj)
            efs_sb = sb.tile([128, ET_SH, D], F32)
            for et in range(ET_SH):
                prj = pp.tile([128, D], F32, tag="ps")
                for i, c in enumerate(DCH):
                    mm(prj[:], efT_sb[:c, i, et * 128:(et + 1) * 128],
                       wproj_sb[:c, i, :], start=(i == 0), stop=(i == 2))
                nc.scalar.mul(efs_sb[:, et, :], efeat_sb[:, et, :],
                              float(alpha))
                nc.vector.scalar_tensor_tensor(
                    ef2_sb[:, et, :], prj[:], float(1.0 - alpha),
                    efs_sb[:, et, :], op0=OP.mult, op1=OP.add)

            ef2T_sb = sb.tile([128, 3, E_SH], F32)
            transpose_512xD(ef2_sb, ef2T_sb)

            # scores (1, 512); locally stabilized exp weights
            sc = pp.tile([1, E_SH], F32, tag="ps")
            for i, c in enumerate(DCH):
                mm(sc[:], ecwatt_sb[:c, i, :], ef2T_sb[:c, i, :],
                   start=(i == 0), stop=(i == 2))
            one_sb = sb.tile([1, 520], F32)
            nloc = one_sb[:, 512:513]
            nc.vector.tensor_reduce(nloc, sc[:], axis=AX.X, op=OP.max,
                                    negate=True)
            expw = one_sb[:, 0:512]
            zloc = one_sb[:, 513:514]
            nc.scalar.activation(expw, sc[:], AF.Exp, bias=nloc, scale=1.0,
                                 accum_out=zloc)
            mloc = one_sb[:, 514:515]
            nc.scalar.mul(mloc, nloc, -1.0)

            expcol_sb = sb.tile([128, ET_SH], F32)
            for et in range(ET_SH):
                tc1 = pp.tile([128, 1], F32, tag="ps")
                nc.tensor.transpose(tc1[:],
                                    expw[0:1, et * 128:(et + 1) * 128],
                                    ident[0:1, 0:1])
                nc.scalar.copy(expcol_sb[:, et:et + 1], tc1[:])

            # G = ef2 @ ec_W_proj ; p2 = expw^T @ G (pooling + proj folded)
            g_sb = sb.tile([128, ET_SH, D], F32)
            for et in range(ET_SH):
                g = pp.tile([128, D], F32, tag="ps")
                for i, c in enumerate(DCH):
                    mm(g[:], ef2T_sb[:c, i, et * 128:(et + 1) * 128],
                       ecwproj_sb[:c, i, :], start=(i == 0), stop=(i == 2))
                nc.scalar.copy(g_sb[:, et, :], g[:])
            p2 = pp.tile([1, D], F32, tag="acc")
            for et in range(ET_SH):
                mm(p2[:], expcol_sb[:, et:et + 1], g_sb[:, et, :],
                   start=(et == 0), stop=(et == ET_SH - 1))

            pk_sb = sb.tile([1, 304], F32)
            nc.scalar.copy(pk_sb[:, 0:D], p2[:])
            nc.scalar.copy(pk_sb[:, 300:301], zloc)
            nc.scalar.copy(pk_sb[:, 301:302], mloc)
            nc.vector.memset(pk_sb[:, 302:304], 0.0)
            nc.sync.dma_start(pk_dram[:], pk_sb[0:1, :])

            # ---------- phase 4: AllGather + redundant epilogue ----------
            nc.gpsimd.collective_compute(
                "AllGather", OP.bypass, replica_groups=groups,
                ins=[pk_dram.opt()], outs=[gath.opt()])

            grow = sb.tile([1, NCORES, 304], F32)
            nc.sync.dma_start(
                grow[:], gath[:].rearrange("c k -> (c k)").rearrange(
                    "(o c k) -> o c k", o=1, c=NCORES))
            g8 = sb.tile([NCORES, 304], F32)
            nc.sync.dma_start(g8[:], gath[:])

            eps_sb = sb.tile([1, 16], F32)
            ngmax = eps_sb[:, 0:1]
            nc.vector.tensor_reduce(ngmax, grow[:, :, 301], axis=AX.X,
                                    op=OP.max, negate=True)
            scal_row = eps_sb[:, 1:9]
            nc.scalar.activation(scal_row, grow[:, :, 301], AF.Exp,
                                 bias=ngmax, scale=1.0)
            sccol = pp.tile([NCORES, 1], F32, tag="ps")
            nc.tensor.transpose(sccol[:], scal_row, ident[0:1, 0:1])
            sccol_sb = sb.tile([NCORES, 1], F32)
            nc.scalar.copy(sccol_sb[:], sccol[:])
            comb = pp.tile([1, 304], F32, tag="ps")
            nc.tensor.matmul(comb[:], sccol_sb[:], g8[:], start=True,
                             stop=True)
            rz = eps_sb[:, 9:10]
            nc.vector.reciprocal(rz, comb[:, 300:301])
            pooled_sb = sb.tile([1, D], F32)
            nc.vector.tensor_scalar_mul(pooled_sb[:], comb[:, 0:D], rz)
            nc.vector.tensor_add(pooled_sb[:], pooled_sb[:], ecb_sb[:])

            ocol_sb = sb.tile([128, 3], F32)
            for i, (c, o) in enumerate(zip(DCH, DOF)):
                tpc = pp.tile([128, 1], F32, tag="ps")
                nc.tensor.transpose(tpc[:c, :], pooled_sb[0:1, o:o + c],
                                    ident[0:1, 0:1])
                nc.scalar.copy(ocol_sb[:c, i:i + 1], tpc[:c, :])
            lg = pp.tile([1, NCAT], F32, tag="acc")
            for i, c in enumerate(DCH):
                nc.tensor.matmul(lg[:], ocol_sb[:c, i:i + 1],
                                 fcw_sb[:c, i, :], start=(i == 0),
                                 stop=(i == 2))
            logit_sb = sb.tile([1, NCAT], F32)
            nc.vector.tensor_add(logit_sb[:], lg[:], fcb_sb[:])
            nc.sync.dma_start(out_d[:], logit_sb[:])

    nc.compile()
    return nc


_CACHE = {}


def get_nc(alpha: float, mode: str = "f32r"):
    key = (alpha, mode)
    if key not in _CACHE:
        _CACHE[key] = _build(alpha, mode)
    return _CACHE[key]


def make_in_maps(node_feats, edge_feats, inc_mat, W_att, W_proj,
                 ec_W_att, ec_W_proj, ec_b_proj, fc_W, fc_b, mode="f32r"):
    cc = lambda a: np.ascontiguousarray(np.asarray(a, np.float32))
    node_feats, inc_mat, edge_feats = cc(node_feats), cc(inc_mat), cc(edge_feats)
    if mode == "bf16":
        import ml_dtypes
        node_feats = node_feats.astype(ml_dtypes.bfloat16)
        inc_mat = inc_mat.astype(ml_dtypes.bfloat16)
    common = dict(watt=cc(W_att), wproj=cc(W_proj),
                  ecwatt=cc(ec_W_att).reshape(D, 1), ecwproj=cc(ec_W_proj),
                  ecb=cc(ec_b_proj), fcw=cc(fc_W), fcb=cc(fc_b))
    in_maps = []
    for c in range(NCORES):
        # under chunked RS, core c owns edges {1024k + 128c .. +128} k=0..3
        eidx = np.concatenate([np.arange(1024 * k + 128 * c,
                                         1024 * k + 128 * (c + 1))
                               for k in range(4)])
        in_maps.append(dict(
            x=node_feats[c * M_SH:(c + 1) * M_SH],
            inc=np.ascontiguousarray(inc_mat[c * M_SH:(c + 1) * M_SH]),
            efeat=np.ascontiguousarray(edge_feats[eidx]),
            **common))
    return in_maps


def kernel(node_feats, edge_feats, inc_mat, W_att, W_proj, alpha,
           ec_W_att, ec_W_proj, ec_b_proj, fc_W, fc_b,
           mode="f32r", trace=False):
    nc = get_nc(float(np.asarray(alpha)), mode)
    in_maps = make_in_maps(node_feats, edge_feats, inc_mat, W_att, W_proj,
                           ec_W_att, ec_W_proj, ec_b_proj, fc_W, fc_b,
                           mode=mode)
    res = run_bass_kernel_spmd(nc, in_maps, list(range(NCORES)), trace=trace)
    kernel.last_results = res
    return res.results[0]["out"].reshape(NCAT).astype(np.float32)



# revision 10
# speedup vs baseline: 1.5137x; 1.5137x over previous
"""HGConv fused kernel for one TRN2 chip (8 NeuronCores), SPMD via Bass/Tile.

Hardcoded for M=16384 nodes, E=4096 hyperedges, D=300, N_CAT=3, 8 cores.

Edge-sharded design (no mid-kernel ReduceScatter):
  - Core c owns edges [512c, 512(c+1)).  It streams the FULL node_feats X
    (fp16, replicated) plus its inc column-slice (fp16, host-pre-tiled so
    every DMA descriptor is a multi-KB contiguous run) and accumulates
    IX[e_c, :] = inc[:, e_c].T @ X locally in 4 PSUM banks (4 e-subtiles
    x 128 edges, contraction over all 128 m-tiles).
  - Tail on the 512 local edges: att = IX @ W_att (reassociated), softmax
    over d (stabilized), ef = IX * attn; ef2T = (1-a)*W_proj.T @ efT +
    a*edge_feats.T (edge_feats pre-transposed/pre-scaled on host);
    scores|G = ef2T.T @ [ec_W_att | ec_W_proj @ fc_W] (classifier weights
    folded on host); unstabilized exp(score) (scores are O(5), fp32-safe);
    p2|z accumulated with a PE matmul against [G | ones].
  - AllGather of the per-core 8-float partials; every core redundantly
    combines (sum / z) and adds the folded bias to produce the (3,) logits.
"""

import sys

for _p in ("/opt/trn_rl_repo", "/opt/pypackages"):
    if _p not in sys.path:
        sys.path.append(_p)

import numpy as np

import concourse.bacc as bacc
import concourse.tile as tile
from concourse import masks, mybir
from concourse.bass_utils import run_bass_kernel_spmd

F32 = mybir.dt.float32
F32R = mybir.dt.float32r
F16 = mybir.dt.float16
AX = mybir.AxisListType
OP = mybir.AluOpType
AF = mybir.ActivationFunctionType

NCORES = 8
M, E, D, NCAT = 16384, 4096, 300, 3
E_SH = E // NCORES          # 512 edges per core
ET_SH = E_SH // 128         # 4 e-subtiles per core
MT_TOT = M // 128           # 128 m-tiles over the full node axis
MCH = 8                     # m-tiles per streamed chunk
NCH = MT_TOT // MCH         # 16 chunks
DCH = (128, 128, 44)        # d split into partition chunks
DOF = (0, 128, 256)


def _build():
    nc = bacc.Bacc("TRN2", target_bir_lowering=False, debug=False,
                   num_devices=NCORES)
    x_d = nc.dram_tensor("x", [128, MT_TOT, D], F16, kind="ExternalInput")
    inc_d = nc.dram_tensor("inc", [128, MT_TOT, E_SH], F16,
                           kind="ExternalInput")
    efT_d = nc.dram_tensor("efT", [128, 3, E_SH], F32, kind="ExternalInput")
    watt_d = nc.dram_tensor("watt", [D, D], F32, kind="ExternalInput")
    wproj_d = nc.dram_tensor("wproj", [D, D], F32, kind="ExternalInput")
    sgw_d = nc.dram_tensor("sgw", [D, 4], F32, kind="ExternalInput")
    b2_d = nc.dram_tensor("b2", [NCAT], F32, kind="ExternalInput")
    out_d = nc.dram_tensor("out", [1, NCAT], F32, kind="ExternalOutput")

    groups = [list(range(NCORES))]

    def mm(out, lhsT, rhs, start, stop):
        nc.tensor.matmul(out, lhsT, rhs, start=start, stop=stop)

    def r(ap):  # reinterpret f32 data as f32r for full-rate matmul
        return ap.bitcast(F32R)

    with tile.TileContext(nc) as tc, \
         tc.tile_pool(name="sb", bufs=1) as sb, \
         tc.tile_pool(name="dram", bufs=1, space="DRAM") as dram:

        pk_dram = dram.tile([8], F32)           # AG input
        gath = dram.tile([NCORES, 8], F32)      # AG output

        # ---------- phase 1: IX[e_c, :] = inc_c.T @ X (full m) ----------
        with tc.tile_pool(name="pacc", bufs=ET_SH, space="PSUM") as pacc, \
             tc.tile_pool(name="xp", bufs=4) as xp, \
             tc.tile_pool(name="ip", bufs=4) as ip:
            acc = [pacc.tile([128, D], F32, name=f"acc{es}", tag="acc")
                   for es in range(ET_SH)]
            for ch in range(NCH):
                mt0 = ch * MCH
                i_t = ip.tile([128, MCH, E_SH], F16, tag="inc",
                              name=f"inc_c{ch}")
                x_t = xp.tile([128, MCH, D], F16, tag="x", name=f"x_c{ch}")
                nc.sync.dma_start(i_t[:], inc_d[:, mt0:mt0 + MCH, :])
                nc.scalar.dma_start(x_t[:], x_d[:, mt0:mt0 + MCH, :])
                for mt in range(MCH):
                    for es in range(ET_SH):
                        mm(acc[es][:],
                           i_t[:, mt, es * 128:(es + 1) * 128],
                           x_t[:, mt, :],
                           start=(ch == 0 and mt == 0),
                           stop=(ch == NCH - 1 and mt == MCH - 1))

            # ---------- small weights / constants (overlap phase 1) ------
            watt_sb = sb.tile([128, 3, D], F32R)
            wproj_sb = sb.tile([128, 3, D], F32R)
            sgw_sb = sb.tile([128, 3, 4], F32R)
            for i, (c, o) in enumerate(zip(DCH, DOF)):
                nc.gpsimd.dma_start(watt_sb[:c, i, :],
                                    watt_d[o:o + c, :].bitcast(F32R))
                nc.gpsimd.dma_start(wproj_sb[:c, i, :],
                                    wproj_d[o:o + c, :].bitcast(F32R))
                nc.gpsimd.dma_start(sgw_sb[:c, i, :],
                                    sgw_d[o:o + c, :].bitcast(F32R))
            b2_sb = sb.tile([1, NCAT], F32)
            nc.gpsimd.dma_start(b2_sb[:],
                                b2_d.ap().rearrange("(o d) -> o d", o=1))
            efT_sb = sb.tile([128, 3, E_SH], F32)
            nc.gpsimd.dma_start(efT_sb[:], efT_d[:])
            ident = sb.tile([128, 128], F32)
            masks.make_identity(nc, ident[:])

            # IX psum -> sbuf (inside pacc scope, then release its banks)
            def cp(k, dst, src):
                # psum-reading copies: only ACT/DVE may touch PSUM
                e = (nc.scalar.copy, nc.vector.tensor_copy)[k % 2]
                e(dst, src)

            ix_sb = sb.tile([128, ET_SH, D], F32)
            for es in range(ET_SH):
                cp(es, ix_sb[:, es, :], acc[es][:])

        # ---------- tail on this core's 512 edges ----------
        if True:
            with tc.tile_pool(name="pp", bufs=4, space="PSUM") as pp:
                def transpose_512xD(src_sb, dstT_sb):
                    # src (128, 4, 300) [e-part] -> dstT (128, 3, 512) [d-part]
                    k = 0
                    for et in range(ET_SH):
                        for i, (c, o) in enumerate(zip(DCH, DOF)):
                            tp = pp.tile([128, 128], F32, tag="ps")
                            nc.tensor.transpose(tp[:c, :128],
                                                src_sb[:, et, o:o + c],
                                                ident[:])
                            cp(k, dstT_sb[:c, i, et * 128:(et + 1) * 128],
                               tp[:c, :128])
                            k += 1

                ixT_sb = sb.tile([128, 3, E_SH], F32R)
                transpose_512xD(ix_sb, ixT_sb)

                # edge_att = IX @ W_att; softmax over d; ef = IX * attn
                ef_sb = sb.tile([128, ET_SH, D], F32)
                stat_sb = sb.tile([128, ET_SH, 4], F32)
                for et in range(ET_SH):
                    att = pp.tile([128, D], F32, tag="ps")
                    for i, c in enumerate(DCH):
                        mm(att[:], ixT_sb[:c, i, et * 128:(et + 1) * 128],
                           watt_sb[:c, i, :], start=(i == 0), stop=(i == 2))
                    nmax = stat_sb[:, et, 0:1]
                    nc.vector.tensor_reduce(nmax, att[:], axis=AX.X,
                                            op=OP.max, negate=True)
                    ex = pp.tile([128, D], F32, tag="ps")
                    rsum = stat_sb[:, et, 1:2]
                    nc.scalar.activation(ex[:], att[:], AF.Exp, bias=nmax,
                                         scale=1.0, accum_out=rsum)
                    rcp = stat_sb[:, et, 2:3]
                    nc.vector.reciprocal(rcp, rsum)
                    nc.vector.scalar_tensor_tensor(
                        ef_sb[:, et, :], ex[:], rcp, ix_sb[:, et, :],
                        op0=OP.mult, op1=OP.mult)

                efTT_sb = sb.tile([128, 3, E_SH], F32R)
                transpose_512xD(ef_sb, efTT_sb)

                # ef2T = (1-a)*W_proj.T @ efT + a*edge_feats.T
                ef2T_sb = sb.tile([128, 3, E_SH], F32R)
                for i, (c, o) in enumerate(zip(DCH, DOF)):
                    pj = pp.tile([128, E_SH], F32, tag="ps")
                    for j, cj in enumerate(DCH):
                        mm(pj[:c, :], wproj_sb[:cj, j, o:o + c],
                           efTT_sb[:cj, j, :], start=(j == 0),
                           stop=(j == 2))
                    nc.vector.tensor_add(ef2T_sb[:c, i, :], pj[:c, :],
                                         efT_sb[:c, i, :])

                # scores|G = ef2 @ [ec_W_att | ec_W_proj @ fc_W]  -> (e, 4)
                g_sb = sb.tile([128, ET_SH, 8], F32)
                nc.vector.memset(g_sb[:, :, 4:5], 1.0)
                expw_sb = sb.tile([128, ET_SH], F32)
                for et in range(ET_SH):
                    sg = pp.tile([128, 4], F32, tag="ps")
                    for j, cj in enumerate(DCH):
                        mm(sg[:], ef2T_sb[:cj, j, et * 128:(et + 1) * 128],
                           sgw_sb[:cj, j, :], start=(j == 0), stop=(j == 2))
                    nc.scalar.copy(g_sb[:, et, 0:4], sg[:])
                    nc.scalar.activation(expw_sb[:, et:et + 1],
                                         g_sb[:, et, 0:1], AF.Exp, scale=1.0)

                # p2|z = sum_e exp_e * [G_e | 1]   (PE contraction over e)
                p2 = pp.tile([1, 4], F32, tag="p2")
                for et in range(ET_SH):
                    mm(p2[:], expw_sb[:, et:et + 1], g_sb[:, et, 1:5],
                       start=(et == 0), stop=(et == ET_SH - 1))
                pk_sb = sb.tile([1, 8], F32)
                nc.vector.memset(pk_sb[:, 4:8], 0.0)
                nc.scalar.copy(pk_sb[:, 0:4], p2[:])
                nc.sync.dma_start(pk_dram[:], pk_sb[0:1, :])

                # ---------- AllGather + redundant epilogue ----------
                nc.gpsimd.collective_compute(
                    "AllGather", OP.bypass, replica_groups=groups,
                    ins=[pk_dram.opt()], outs=[gath.opt()])

                g8 = sb.tile([NCORES, 8], F32)
                nc.sync.dma_start(g8[:], gath[:])
                ones8 = sb.tile([NCORES, 1], F32)
                nc.vector.memset(ones8[:], 1.0)
                comb = pp.tile([1, 8], F32, tag="ps")
                nc.tensor.matmul(comb[:], ones8[:], g8[:], start=True,
                                 stop=True)
                rz = sb.tile([1, 1], F32)
                nc.vector.reciprocal(rz[:], comb[:, 3:4])
                lg_sb = sb.tile([1, NCAT], F32)
                nc.vector.scalar_tensor_tensor(
                    lg_sb[:], comb[:, 0:3], rz[:], b2_sb[:],
                    op0=OP.mult, op1=OP.add)
                nc.sync.dma_start(out_d[:], lg_sb[:])

    nc.compile()
    return nc


_CACHE = {}


def get_nc():
    if "nc" not in _CACHE:
        _CACHE["nc"] = _build()
    return _CACHE["nc"]


def make_in_maps(node_feats, edge_feats, inc_mat, W_att, W_proj, alpha,
                 ec_W_att, ec_W_proj, ec_b_proj, fc_W, fc_b):
    cc = lambda a: np.ascontiguousarray(np.asarray(a, np.float32))
    node_feats = cc(node_feats)
    inc_mat = cc(inc_mat)
    edge_feats = cc(edge_feats)
    W_att, W_proj = cc(W_att), cc(W_proj)
    ec_W_att, ec_W_proj = cc(ec_W_att).reshape(D, 1), cc(ec_W_proj)
    ec_b_proj, fc_W, fc_b = cc(ec_b_proj), cc(fc_W), cc(fc_b)
    a = float(np.asarray(alpha))

    # x packed [p, mt, d] fp16, replicated
    x_pack = np.ascontiguousarray(
        node_feats.reshape(MT_TOT, 128, D).transpose(1, 0, 2)
    ).astype(np.float16)
    # folded weights
    G2 = ec_W_proj @ fc_W                     # (300, 3)
    sgw = np.ascontiguousarray(
        np.concatenate([ec_W_att, G2], axis=1))  # (300, 4)
    b2 = ec_b_proj @ fc_W + fc_b              # (3,)
    wproj_s = np.ascontiguousarray((1.0 - a) * W_proj)
    common = dict(x=x_pack, watt=W_att, wproj=wproj_s, sgw=sgw, b2=b2)

    in_maps = []
    for c in range(NCORES):
        sl = slice(c * E_SH, (c + 1) * E_SH)
        inc_pack = np.ascontiguousarray(
            inc_mat[:, sl].reshape(MT_TOT, 128, E_SH).transpose(1, 0, 2)
        ).astype(np.float16)
        efT = np.zeros((128, 3, E_SH), np.float32)
        eft_full = a * edge_feats[sl].T       # (300, 512), pre-scaled
        for i, (cch, o) in enumerate(zip(DCH, DOF)):
            efT[:cch, i, :] = eft_full[o:o + cch, :]
        in_maps.append(dict(inc=inc_pack, efT=efT, **common))
    return in_maps


def kernel(node_feats, edge_feats, inc_mat, W_att, W_proj, alpha,
           ec_W_att, ec_W_proj, ec_b_proj, fc_W, fc_b, trace=False):
    nc = get_nc()
    in_maps = make_in_maps(node_feats, edge_feats, inc_mat, W_att, W_proj,
                           alpha, ec_W_att, ec_W_proj, ec_b_proj, fc_W, fc_b)
    res = run_bass_kernel_spmd(nc, in_maps, list(range(NCORES)), trace=trace)
    kernel.last_results = res
    return res.results[0]["out"].reshape(NCAT).astype(np.float32)


# revision 12
# speedup vs baseline: 1.5177x; 1.0026x over previous
"""HGConv fused kernel for one TRN2 chip (8 NeuronCores), SPMD via Bass/Tile.

Hardcoded for M=16384 nodes, E=4096 hyperedges, D=300, N_CAT=3, 8 cores.

Edge-sharded design (no mid-kernel ReduceScatter):
  - Core c owns edges [512c, 512(c+1)).  It streams the FULL node_feats X
    (fp16, replicated) plus its inc column-slice (fp16, host-pre-tiled so
    every DMA descriptor is a multi-KB contiguous run) and accumulates
    IX[e_c, :] = inc[:, e_c].T @ X locally in 4 PSUM banks (4 e-subtiles
    x 128 edges, contraction over all 128 m-tiles).
  - Tail on the 512 local edges: att = IX @ W_att (reassociated), softmax
    over d (stabilized), ef = IX * attn; ef2T = (1-a)*W_proj.T @ efT +
    a*edge_feats.T (edge_feats pre-transposed/pre-scaled on host);
    scores|G = ef2T.T @ [ec_W_att | ec_W_proj @ fc_W] (classifier weights
    folded on host); unstabilized exp(score) (scores are O(5), fp32-safe);
    p2|z accumulated with a PE matmul against [G | ones].
  - AllGather of the per-core 8-float partials; every core redundantly
    combines (sum / z) and adds the folded bias to produce the (3,) logits.
"""

import sys

for _p in ("/opt/trn_rl_repo", "/opt/pypackages"):
    if _p not in sys.path:
        sys.path.append(_p)

import numpy as np

import concourse.bacc as bacc
import concourse.tile as tile
from concourse import masks, mybir
from concourse.bass_utils import run_bass_kernel_spmd

F32 = mybir.dt.float32
F32R = mybir.dt.float32r
F16 = mybir.dt.float16
AX = mybir.AxisListType
OP = mybir.AluOpType
AF = mybir.ActivationFunctionType

NCORES = 8
M, E, D, NCAT = 16384, 4096, 300, 3
E_SH = E // NCORES          # 512 edges per core
ET_SH = E_SH // 128         # 4 e-subtiles per core
MT_TOT = M // 128           # 128 m-tiles over the full node axis
MCH = 8                     # m-tiles per streamed chunk
NCH = MT_TOT // MCH         # 16 chunks
DCH = (128, 128, 44)        # d split into partition chunks
DOF = (0, 128, 256)


def _build():
    nc = bacc.Bacc("TRN2", target_bir_lowering=False, debug=False,
                   num_devices=NCORES)
    x_d = nc.dram_tensor("x", [128, MT_TOT, D], F16, kind="ExternalInput")
    inc_d = nc.dram_tensor("inc", [128, MT_TOT, E_SH], F16,
                           kind="ExternalInput")
    efT_d = nc.dram_tensor("efT", [128, 3, E_SH], F32, kind="ExternalInput")
    watt_d = nc.dram_tensor("watt", [D, D], F32, kind="ExternalInput")
    wproj_d = nc.dram_tensor("wproj", [D, D], F32, kind="ExternalInput")
    sgw_d = nc.dram_tensor("sgw", [D, 4], F32, kind="ExternalInput")
    b2_d = nc.dram_tensor("b2", [NCAT], F32, kind="ExternalInput")
    out_d = nc.dram_tensor("out", [1, NCAT], F32, kind="ExternalOutput")

    groups = [list(range(NCORES))]

    def mm(out, lhsT, rhs, start, stop):
        nc.tensor.matmul(out, lhsT, rhs, start=start, stop=stop)

    def r(ap):  # reinterpret f32 data as f32r for full-rate matmul
        return ap.bitcast(F32R)

    with tile.TileContext(nc) as tc, \
         tc.tile_pool(name="sb", bufs=1) as sb, \
         tc.tile_pool(name="dram", bufs=1, space="DRAM") as dram:

        pk_dram = dram.tile([8], F32)           # AG input
        gath = dram.tile([NCORES, 8], F32)      # AG output

        # ---------- phase 1: IX[e_c, :] = inc_c.T @ X (full m) ----------
        with tc.tile_pool(name="pacc", bufs=ET_SH, space="PSUM") as pacc, \
             tc.tile_pool(name="xp", bufs=8) as xp, \
             tc.tile_pool(name="ip", bufs=8) as ip:
            acc = [pacc.tile([128, D], F32, name=f"acc{es}", tag="acc")
                   for es in range(ET_SH)]
            for ch in range(NCH):
                mt0 = ch * MCH
                i_t = ip.tile([128, MCH, E_SH], F16, tag="inc",
                              name=f"inc_c{ch}")
                x_t = xp.tile([128, MCH, D], F16, tag="x", name=f"x_c{ch}")
                nc.sync.dma_start(i_t[:], inc_d[:, mt0:mt0 + MCH, :])
                nc.scalar.dma_start(x_t[:], x_d[:, mt0:mt0 + MCH, :])
                for mt in range(MCH):
                    for es in range(ET_SH):
                        mm(acc[es][:],
                           i_t[:, mt, es * 128:(es + 1) * 128],
                           x_t[:, mt, :],
                           start=(ch == 0 and mt == 0),
                           stop=(ch == NCH - 1 and mt == MCH - 1))

            # ---------- small weights / constants (overlap phase 1) ------
            watt_sb = sb.tile([128, 3, D], F32R)
            wproj_sb = sb.tile([128, 3, D], F32R)
            sgw_sb = sb.tile([128, 3, 4], F32R)
            for i, (c, o) in enumerate(zip(DCH, DOF)):
                nc.gpsimd.dma_start(watt_sb[:c, i, :],
                                    watt_d[o:o + c, :].bitcast(F32R))
                nc.gpsimd.dma_start(wproj_sb[:c, i, :],
                                    wproj_d[o:o + c, :].bitcast(F32R))
                nc.gpsimd.dma_start(sgw_sb[:c, i, :],
                                    sgw_d[o:o + c, :].bitcast(F32R))
            b2_sb = sb.tile([1, NCAT], F32)
            nc.gpsimd.dma_start(b2_sb[:],
                                b2_d.ap().rearrange("(o d) -> o d", o=1))
            efT_sb = sb.tile([128, 3, E_SH], F32)
            nc.gpsimd.dma_start(efT_sb[:], efT_d[:])
            ident = sb.tile([128, 128], F32)
            masks.make_identity(nc, ident[:])

            # IX psum -> sbuf (inside pacc scope, then release its banks)
            def cp(k, dst, src):
                # psum-reading copies: only ACT/DVE may touch PSUM
                e = (nc.scalar.copy, nc.vector.tensor_copy)[k % 2]
                e(dst, src)

            ix_sb = sb.tile([128, ET_SH, D], F32)
            for es in range(ET_SH):
                cp(es, ix_sb[:, es, :], acc[es][:])

        # ---------- tail on this core's 512 edges ----------
        if True:
            with tc.tile_pool(name="pp", bufs=4, space="PSUM") as pp:
                def transpose_512xD(src_sb, dstT_sb):
                    # src (128, 4, 300) [e-part] -> dstT (128, 3, 512) [d-part]
                    k = 0
                    for et in range(ET_SH):
                        for i, (c, o) in enumerate(zip(DCH, DOF)):
                            tp = pp.tile([128, 128], F32, tag="ps")
                            nc.tensor.transpose(tp[:c, :128],
                                                src_sb[:, et, o:o + c],
                                                ident[:])
                            cp(k, dstT_sb[:c, i, et * 128:(et + 1) * 128],
                               tp[:c, :128])
                            k += 1

                ixT_sb = sb.tile([128, 3, E_SH], F32R)
                transpose_512xD(ix_sb, ixT_sb)

                # edge_att = IX @ W_att; softmax over d; ef = IX * attn
                ef_sb = sb.tile([128, ET_SH, D], F32)
                stat_sb = sb.tile([128, ET_SH, 4], F32)
                for et in range(ET_SH):
                    att = pp.tile([128, D], F32, tag="ps")
                    for i, c in enumerate(DCH):
                        mm(att[:], ixT_sb[:c, i, et * 128:(et + 1) * 128],
                           watt_sb[:c, i, :], start=(i == 0), stop=(i == 2))
                    nmax = stat_sb[:, et, 0:1]
                    nc.vector.tensor_reduce(nmax, att[:], axis=AX.X,
                                            op=OP.max, negate=True)
                    ex = pp.tile([128, D], F32, tag="ps")
                    rsum = stat_sb[:, et, 1:2]
                    nc.scalar.activation(ex[:], att[:], AF.Exp, bias=nmax,
                                         scale=1.0, accum_out=rsum)
                    rcp = stat_sb[:, et, 2:3]
                    nc.vector.reciprocal(rcp, rsum)
                    nc.vector.scalar_tensor_tensor(
                        ef_sb[:, et, :], ex[:], rcp, ix_sb[:, et, :],
                        op0=OP.mult, op1=OP.mult)

                efTT_sb = sb.tile([128, 3, E_SH], F32R)
                transpose_512xD(ef_sb, efTT_sb)

                # ef2T = (1-a)*W_proj.T @ efT + a*edge_feats.T
                ef2T_sb = sb.tile([128, 3, E_SH], F32R)
                for i, (c, o) in enumerate(zip(DCH, DOF)):
                    pj = pp.tile([128, E_SH], F32, tag="ps")
                    for j, cj in enumerate(DCH):
                        mm(pj[:c, :], wproj_sb[:cj, j, o:o + c],
                           efTT_sb[:cj, j, :], start=(j == 0),
                           stop=(j == 2))
                    nc.vector.tensor_add(ef2T_sb[:c, i, :], pj[:c, :],
                                         efT_sb[:c, i, :])

                # scores|G = ef2 @ [ec_W_att | ec_W_proj @ fc_W]  -> (e, 4)
                g_sb = sb.tile([128, ET_SH, 8], F32)
                nc.vector.memset(g_sb[:, :, 4:5], 1.0)
                expw_sb = sb.tile([128, ET_SH], F32)
                for et in range(ET_SH):
                    sg = pp.tile([128, 4], F32, tag="ps")
                    for j, cj in enumerate(DCH):
                        mm(sg[:], ef2T_sb[:cj, j, et * 128:(et + 1) * 128],
                           sgw_sb[:cj, j, :], start=(j == 0), stop=(j == 2))
                    nc.scalar.copy(g_sb[:, et, 0:4], sg[:])
                    nc.scalar.activation(expw_sb[:, et:et + 1],
                                         g_sb[:, et, 0:1], AF.Exp, scale=1.0)

                # p2|z = sum_e exp_e * [G_e | 1]   (PE contraction over e)
                p2 = pp.tile([1, 4], F32, tag="p2")
                for et in range(ET_SH):
                    mm(p2[:], expw_sb[:, et:et + 1], g_sb[:, et, 1:5],
                       start=(et == 0), stop=(et == ET_SH - 1))
                pk_sb = sb.tile([1, 8], F32)
                nc.vector.memset(pk_sb[:, 4:8], 0.0)
                nc.scalar.copy(pk_sb[:, 0:4], p2[:])
                nc.sync.dma_start(pk_dram[:], pk_sb[0:1, :])

                # ---------- AllGather + redundant epilogue ----------
                nc.gpsimd.collective_compute(
                    "AllGather", OP.bypass, replica_groups=groups,
                    ins=[pk_dram.opt()], outs=[gath.opt()])

                g8 = sb.tile([NCORES, 8], F32)
                nc.sync.dma_start(g8[:], gath[:])
                ones8 = sb.tile([NCORES, 1], F32)
                nc.vector.memset(ones8[:], 1.0)
                comb = pp.tile([1, 8], F32, tag="ps")
                nc.tensor.matmul(comb[:], ones8[:], g8[:], start=True,
                                 stop=True)
                rz = sb.tile([1, 1], F32)
                nc.vector.reciprocal(rz[:], comb[:, 3:4])
                lg_sb = sb.tile([1, NCAT], F32)
                nc.vector.scalar_tensor_tensor(
                    lg_sb[:], comb[:, 0:3], rz[:], b2_sb[:],
                    op0=OP.mult, op1=OP.add)
                nc.sync.dma_start(out_d[:], lg_sb[:])

    nc.compile()
    return nc


_CACHE = {}


def get_nc():
    if "nc" not in _CACHE:
        _CACHE["nc"] = _build()
    return _CACHE["nc"]


def make_in_maps(node_feats, edge_feats, inc_mat, W_att, W_proj, alpha,
                 ec_W_att, ec_W_proj, ec_b_proj, fc_W, fc_b):
    cc = lambda a: np.ascontiguousarray(np.asarray(a, np.float32))
    node_feats = cc(node_feats)
    inc_mat = cc(inc_mat)
    edge_feats = cc(edge_feats)
    W_att, W_proj = cc(W_att), cc(W_proj)
    ec_W_att, ec_W_proj = cc(ec_W_att).reshape(D, 1), cc(ec_W_proj)
    ec_b_proj, fc_W, fc_b = cc(ec_b_proj), cc(fc_W), cc(fc_b)
    a = float(np.asarray(alpha))

    # x packed [p, mt, d] fp16, replicated
    x_pack = np.ascontiguousarray(
        node_feats.reshape(MT_TOT, 128, D).transpose(1, 0, 2)
    ).astype(np.float16)
    # folded weights
    G2 = ec_W_proj @ fc_W                     # (300, 3)
    sgw = np.ascontiguousarray(
        np.concatenate([ec_W_att, G2], axis=1))  # (300, 4)
    b2 = ec_b_proj @ fc_W + fc_b              # (3,)
    wproj_s = np.ascontiguousarray((1.0 - a) * W_proj)
    common = dict(x=x_pack, watt=W_att, wproj=wproj_s, sgw=sgw, b2=b2)

    in_maps = []
    for c in range(NCORES):
        sl = slice(c * E_SH, (c + 1) * E_SH)
        # rotate the m-tile order per core so the 8 cores never stream the
        # same region of the replicated x at the same instant (HBM hotspot)
        rot = np.roll(np.arange(MT_TOT), -c * (MT_TOT // NCORES))
        inc_pack = np.ascontiguousarray(
            inc_mat[:, sl].reshape(MT_TOT, 128, E_SH)[rot].transpose(1, 0, 2)
        ).astype(np.float16)
        x_rot = np.ascontiguousarray(common["x"][:, rot, :])
        efT = np.zeros((128, 3, E_SH), np.float32)
        eft_full = a * edge_feats[sl].T       # (300, 512), pre-scaled
        for i, (cch, o) in enumerate(zip(DCH, DOF)):
            efT[:cch, i, :] = eft_full[o:o + cch, :]
        in_maps.append(dict(inc=inc_pack, efT=efT,
                            **{k: v for k, v in common.items() if k != "x"},
                            x=x_rot))
    return in_maps


def kernel(node_feats, edge_feats, inc_mat, W_att, W_proj, alpha,
           ec_W_att, ec_W_proj, ec_b_proj, fc_W, fc_b, trace=False):
    nc = get_nc()
    in_maps = make_in_maps(node_feats, edge_feats, inc_mat, W_att, W_proj,
                           alpha, ec_W_att, ec_W_proj, ec_b_proj, fc_W, fc_b)
    res = run_bass_kernel_spmd(nc, in_maps, list(range(NCORES)), trace=trace)
    kernel.last_results = res
    return res.results[0]["out"].reshape(NCAT).astype(np.float32)


# revision 14
# speedup vs baseline: 1.6591x; 1.0931x over previous
"""HGConv fused kernel for one TRN2 chip (8 NeuronCores), SPMD via Bass/Tile.

Hardcoded for M=16384 nodes, E=4096 hyperedges, D=300, N_CAT=3, 8 cores.

Edge-sharded design (no mid-kernel ReduceScatter):
  - Core c owns edges [512c, 512(c+1)).  It streams the FULL node_feats X
    (fp16, replicated) plus its inc column-slice (fp16, host-pre-tiled so
    every DMA descriptor is a multi-KB contiguous run) and accumulates
    IX[e_c, :] = inc[:, e_c].T @ X locally in 4 PSUM banks (4 e-subtiles
    x 128 edges, contraction over all 128 m-tiles).
  - Tail on the 512 local edges: att = IX @ W_att (reassociated), softmax
    over d (stabilized), ef = IX * attn; ef2T = (1-a)*W_proj.T @ efT +
    a*edge_feats.T (edge_feats pre-transposed/pre-scaled on host);
    scores|G = ef2T.T @ [ec_W_att | ec_W_proj @ fc_W] (classifier weights
    folded on host); unstabilized exp(score) (scores are O(5), fp32-safe);
    p2|z accumulated with a PE matmul against [G | ones].
  - AllGather of the per-core 8-float partials; every core redundantly
    combines (sum / z) and adds the folded bias to produce the (3,) logits.
"""

import sys

for _p in ("/opt/trn_rl_repo", "/opt/pypackages"):
    if _p not in sys.path:
        sys.path.append(_p)

import numpy as np

import concourse.bacc as bacc
import concourse.tile as tile
from concourse import masks, mybir
from concourse.bass_utils import run_bass_kernel_spmd

F32 = mybir.dt.float32
F32R = mybir.dt.float32r
F16 = mybir.dt.float16
AX = mybir.AxisListType
OP = mybir.AluOpType
AF = mybir.ActivationFunctionType

NCORES = 8
M, E, D, NCAT = 16384, 4096, 300, 3
E_SH = E // NCORES          # 512 edges per core
ET_SH = E_SH // 128         # 4 e-subtiles per core
MT_TOT = M // 128           # 128 m-tiles over the full node axis
MCH = 8                     # m-tiles per streamed chunk
NCH = MT_TOT // MCH         # 16 chunks
DCH = (128, 128, 44)        # d split into partition chunks
DOF = (0, 128, 256)


def _build():
    nc = bacc.Bacc("TRN2", target_bir_lowering=False, debug=False,
                   num_devices=NCORES)
    x_d = nc.dram_tensor("x", [128, MT_TOT, D], F16, kind="ExternalInput")
    inc_d = nc.dram_tensor("inc", [128, MT_TOT, E_SH], F16,
                           kind="ExternalInput")
    efT_d = nc.dram_tensor("efT", [128, 3, E_SH], F32, kind="ExternalInput")
    watt_d = nc.dram_tensor("watt", [D, D], F32, kind="ExternalInput")
    wproj_d = nc.dram_tensor("wproj", [D, D], F32, kind="ExternalInput")
    sgw_d = nc.dram_tensor("sgw", [D, 4], F32, kind="ExternalInput")
    b2_d = nc.dram_tensor("b2", [NCAT], F32, kind="ExternalInput")
    out_d = nc.dram_tensor("out", [1, NCAT], F32, kind="ExternalOutput")

    groups = [list(range(NCORES))]

    def mm(out, lhsT, rhs, start, stop):
        nc.tensor.matmul(out, lhsT, rhs, start=start, stop=stop)

    def r(ap):  # reinterpret f32 data as f32r for full-rate matmul
        return ap.bitcast(F32R)

    with tile.TileContext(nc) as tc, \
         tc.tile_pool(name="sb", bufs=1) as sb, \
         tc.tile_pool(name="dram", bufs=1, space="DRAM") as dram:

        pk_dram = dram.tile([8], F32)           # AG input
        gath = dram.tile([NCORES, 8], F32)      # AG output
        wrm_in = dram.tile([8], F32)            # CC warm-up (garbage data)
        wrm_out = dram.tile([NCORES, 8], F32)

        # ---------- phase 1: IX[e_c, :] = inc_c.T @ X (full m) ----------
        with tc.tile_pool(name="pacc", bufs=ET_SH, space="PSUM") as pacc, \
             tc.tile_pool(name="xp", bufs=8) as xp, \
             tc.tile_pool(name="ip", bufs=8) as ip:
            acc = [pacc.tile([128, D], F32, name=f"acc{es}", tag="acc")
                   for es in range(ET_SH)]
            # fire a dummy collective immediately: the first collective of a
            # kernel pays ~11us of CC-engine spin-up; this one absorbs it
            # (and the core-launch skew) while phase 1 runs, so the real
            # AllGather at the end starts in ~1us.
            nc.gpsimd.collective_compute(
                "AllGather", OP.bypass, replica_groups=groups,
                ins=[wrm_in.opt()], outs=[wrm_out.opt()])
            for ch in range(NCH):
                mt0 = ch * MCH
                i_t = ip.tile([128, MCH, E_SH], F16, tag="inc",
                              name=f"inc_c{ch}")
                x_t = xp.tile([128, MCH, D], F16, tag="x", name=f"x_c{ch}")
                nc.sync.dma_start(i_t[:], inc_d[:, mt0:mt0 + MCH, :])
                nc.scalar.dma_start(x_t[:], x_d[:, mt0:mt0 + MCH, :])
                for mt in range(MCH):
                    for es in range(ET_SH):
                        mm(acc[es][:],
                           i_t[:, mt, es * 128:(es + 1) * 128],
                           x_t[:, mt, :],
                           start=(ch == 0 and mt == 0),
                           stop=(ch == NCH - 1 and mt == MCH - 1))

            # ---------- small weights / constants (overlap phase 1) ------
            watt_sb = sb.tile([128, 3, D], F32R)
            wproj_sb = sb.tile([128, 3, D], F32R)
            sgw_sb = sb.tile([128, 3, 4], F32R)
            for i, (c, o) in enumerate(zip(DCH, DOF)):
                nc.gpsimd.dma_start(watt_sb[:c, i, :],
                                    watt_d[o:o + c, :].bitcast(F32R))
                nc.gpsimd.dma_start(wproj_sb[:c, i, :],
                                    wproj_d[o:o + c, :].bitcast(F32R))
                nc.gpsimd.dma_start(sgw_sb[:c, i, :],
                                    sgw_d[o:o + c, :].bitcast(F32R))
            b2_sb = sb.tile([1, NCAT], F32)
            nc.gpsimd.dma_start(b2_sb[:],
                                b2_d.ap().rearrange("(o d) -> o d", o=1))
            efT_sb = sb.tile([128, 3, E_SH], F32)
            nc.gpsimd.dma_start(efT_sb[:], efT_d[:])
            ident = sb.tile([128, 128], F32)
            masks.make_identity(nc, ident[:])

            # IX psum -> sbuf (inside pacc scope, then release its banks)
            def cp(k, dst, src):
                # psum-reading copies: only ACT/DVE may touch PSUM
                e = (nc.scalar.copy, nc.vector.tensor_copy)[k % 2]
                e(dst, src)

            ix_sb = sb.tile([128, ET_SH, D], F32)
            for es in range(ET_SH):
                cp(es, ix_sb[:, es, :], acc[es][:])

        # ---------- tail on this core's 512 edges ----------
        if True:
            with tc.tile_pool(name="pp", bufs=4, space="PSUM") as pp:
                def transpose_512xD(src_sb, dstT_sb):
                    # src (128, 4, 300) [e-part] -> dstT (128, 3, 512) [d-part]
                    k = 0
                    for et in range(ET_SH):
                        for i, (c, o) in enumerate(zip(DCH, DOF)):
                            tp = pp.tile([128, 128], F32, tag="ps")
                            nc.tensor.transpose(tp[:c, :128],
                                                src_sb[:, et, o:o + c],
                                                ident[:])
                            cp(k, dstT_sb[:c, i, et * 128:(et + 1) * 128],
                               tp[:c, :128])
                            k += 1

                ixT_sb = sb.tile([128, 3, E_SH], F32R)
                transpose_512xD(ix_sb, ixT_sb)

                # edge_att = IX @ W_att; softmax over d; ef = IX * attn
                ef_sb = sb.tile([128, ET_SH, D], F32)
                stat_sb = sb.tile([128, ET_SH, 4], F32)
                for et in range(ET_SH):
                    att = pp.tile([128, D], F32, tag="ps")
                    for i, c in enumerate(DCH):
                        mm(att[:], ixT_sb[:c, i, et * 128:(et + 1) * 128],
                           watt_sb[:c, i, :], start=(i == 0), stop=(i == 2))
                    nmax = stat_sb[:, et, 0:1]
                    nc.vector.tensor_reduce(nmax, att[:], axis=AX.X,
                                            op=OP.max, negate=True)
                    ex = pp.tile([128, D], F32, tag="ps")
                    rsum = stat_sb[:, et, 1:2]
                    nc.scalar.activation(ex[:], att[:], AF.Exp, bias=nmax,
                                         scale=1.0, accum_out=rsum)
                    rcp = stat_sb[:, et, 2:3]
                    nc.vector.reciprocal(rcp, rsum)
                    nc.vector.scalar_tensor_tensor(
                        ef_sb[:, et, :], ex[:], rcp, ix_sb[:, et, :],
                        op0=OP.mult, op1=OP.mult)

                efTT_sb = sb.tile([128, 3, E_SH], F32R)
                transpose_512xD(ef_sb, efTT_sb)

                # ef2T = (1-a)*W_proj.T @ efT + a*edge_feats.T
                ef2T_sb = sb.tile([128, 3, E_SH], F32R)
                for i, (c, o) in enumerate(zip(DCH, DOF)):
                    pj = pp.tile([128, E_SH], F32, tag="ps")
                    for j, cj in enumerate(DCH):
                        mm(pj[:c, :], wproj_sb[:cj, j, o:o + c],
                           efTT_sb[:cj, j, :], start=(j == 0),
                           stop=(j == 2))
                    nc.vector.tensor_add(ef2T_sb[:c, i, :], pj[:c, :],
                                         efT_sb[:c, i, :])

                # scores|G = ef2 @ [ec_W_att | ec_W_proj @ fc_W]  -> (e, 4)
                g_sb = sb.tile([128, ET_SH, 8], F32)
                nc.vector.memset(g_sb[:, :, 4:5], 1.0)
                expw_sb = sb.tile([128, ET_SH], F32)
                for et in range(ET_SH):
                    sg = pp.tile([128, 4], F32, tag="ps")
                    for j, cj in enumerate(DCH):
                        mm(sg[:], ef2T_sb[:cj, j, et * 128:(et + 1) * 128],
                           sgw_sb[:cj, j, :], start=(j == 0), stop=(j == 2))
                    nc.scalar.copy(g_sb[:, et, 0:4], sg[:])
                    nc.scalar.activation(expw_sb[:, et:et + 1],
                                         g_sb[:, et, 0:1], AF.Exp, scale=1.0)

                # p2|z = sum_e exp_e * [G_e | 1]   (PE contraction over e)
                p2 = pp.tile([1, 4], F32, tag="p2")
                for et in range(ET_SH):
                    mm(p2[:], expw_sb[:, et:et + 1], g_sb[:, et, 1:5],
                       start=(et == 0), stop=(et == ET_SH - 1))
                pk_sb = sb.tile([1, 8], F32)
                nc.vector.memset(pk_sb[:, 4:8], 0.0)
                nc.scalar.copy(pk_sb[:, 0:4], p2[:])
                nc.sync.dma_start(pk_dram[:], pk_sb[0:1, :])

                # ---------- AllGather + redundant epilogue ----------
                nc.gpsimd.collective_compute(
                    "AllGather", OP.bypass, replica_groups=groups,
                    ins=[pk_dram.opt()], outs=[gath.opt()])

                g8 = sb.tile([NCORES, 8], F32)
                nc.sync.dma_start(g8[:], gath[:])
                ones8 = sb.tile([NCORES, 1], F32)
                nc.vector.memset(ones8[:], 1.0)
                comb = pp.tile([1, 8], F32, tag="ps")
                nc.tensor.matmul(comb[:], ones8[:], g8[:], start=True,
                                 stop=True)
                rz = sb.tile([1, 1], F32)
                nc.vector.reciprocal(rz[:], comb[:, 3:4])
                lg_sb = sb.tile([1, NCAT], F32)
                nc.vector.scalar_tensor_tensor(
                    lg_sb[:], comb[:, 0:3], rz[:], b2_sb[:],
                    op0=OP.mult, op1=OP.add)
                nc.sync.dma_start(out_d[:], lg_sb[:])

    nc.compile()
    return nc


_CACHE = {}


def get_nc():
    if "nc" not in _CACHE:
        _CACHE["nc"] = _build()
    return _CACHE["nc"]


def make_in_maps(node_feats, edge_feats, inc_mat, W_att, W_proj, alpha,
                 ec_W_att, ec_W_proj, ec_b_proj, fc_W, fc_b):
    cc = lambda a: np.ascontiguousarray(np.asarray(a, np.float32))
    node_feats = cc(node_feats)
    inc_mat = cc(inc_mat)
    edge_feats = cc(edge_feats)
    W_att, W_proj = cc(W_att), cc(W_proj)
    ec_W_att, ec_W_proj = cc(ec_W_att).reshape(D, 1), cc(ec_W_proj)
    ec_b_proj, fc_W, fc_b = cc(ec_b_proj), cc(fc_W), cc(fc_b)
    a = float(np.asarray(alpha))

    # x packed [p, mt, d] fp16, replicated
    x_pack = np.ascontiguousarray(
        node_feats.reshape(MT_TOT, 128, D).transpose(1, 0, 2)
    ).astype(np.float16)
    # folded weights
    G2 = ec_W_proj @ fc_W                     # (300, 3)
    sgw = np.ascontiguousarray(
        np.concatenate([ec_W_att, G2], axis=1))  # (300, 4)
    b2 = ec_b_proj @ fc_W + fc_b              # (3,)
    wproj_s = np.ascontiguousarray((1.0 - a) * W_proj)
    common = dict(x=x_pack, watt=W_att, wproj=wproj_s, sgw=sgw, b2=b2)

    in_maps = []
    for c in range(NCORES):
        sl = slice(c * E_SH, (c + 1) * E_SH)
        # rotate the m-tile order per core so the 8 cores never stream the
        # same region of the replicated x at the same instant (HBM hotspot)
        rot = np.roll(np.arange(MT_TOT), -c * (MT_TOT // NCORES))
        inc_pack = np.ascontiguousarray(
            inc_mat[:, sl].reshape(MT_TOT, 128, E_SH)[rot].transpose(1, 0, 2)
        ).astype(np.float16)
        x_rot = np.ascontiguousarray(common["x"][:, rot, :])
        efT = np.zeros((128, 3, E_SH), np.float32)
        eft_full = a * edge_feats[sl].T       # (300, 512), pre-scaled
        for i, (cch, o) in enumerate(zip(DCH, DOF)):
            efT[:cch, i, :] = eft_full[o:o + cch, :]
        in_maps.append(dict(inc=inc_pack, efT=efT,
                            **{k: v for k, v in common.items() if k != "x"},
                            x=x_rot))
    return in_maps


def kernel(node_feats, edge_feats, inc_mat, W_att, W_proj, alpha,
           ec_W_att, ec_W_proj, ec_b_proj, fc_W, fc_b, trace=False):
    nc = get_nc()
    in_maps = make_in_maps(node_feats, edge_feats, inc_mat, W_att, W_proj,
                           alpha, ec_W_att, ec_W_proj, ec_b_proj, fc_W, fc_b)
    res = run_bass_kernel_spmd(nc, in_maps, list(range(NCORES)), trace=trace)
    kernel.last_results = res
    return res.results[0]["out"].reshape(NCAT).astype(np.float32)
